# revision 6
# baseline (speedup 1.0000x reference)
"""CondConv kernel for Trainium2 (8 NeuronCores, data-parallel over batch).

Computation (per sample b):
  att   = sigmoid(mean_hw(x_b) @ att_w.T)                       [K]
  agg_w = sum_k att[k] * weight[k]    (3x3 conv weights, O,I)   [O,I,3,3]
  out   = BN(conv2d(x_b, agg_w, pad=1) + att @ bias) -> relu    [O,H,W]

Strategy: pure data parallel, 4 samples per core.  The conv runs as
9 shifted bf16 matmuls (contraction over C_in on partitions, 2 c-tiles)
accumulating in PSUM; spatial dim tiled in 7 row-blocks of 8 rows
(N=448) which share each loaded weight tile.  BN scale is folded into
the weights on the host; BN shift + conv bias fold into the PSUM drain
(relu(psum + bias_tot)).  Next-sample prep (x load / cast+pool / att /
weight-combine) is interleaved mid-conv so the PE never idles at sample
boundaries.
"""

from contextlib import ExitStack
from itertools import product

import ml_dtypes
import numpy as np

import concourse.bass as bass
import concourse.mybir as mybir
import concourse.tile as tile
from concourse import bacc, bass_isa
from concourse.bass_utils import run_bass_kernel_spmd

BS, C_IN, H, W = 32, 256, 56, 56
C_OUT, K_EXP = 256, 4
NCORES = 8
BPC = BS // NCORES          # samples per core
CT = C_IN // 128            # input-channel tiles
OT = C_OUT // 128           # output-channel tiles
R = 8                       # output rows per psum block

F32 = mybir.dt.float32
BF16 = mybir.dt.bfloat16
AF = mybir.ActivationFunctionType
ALU = mybir.AluOpType


def build_program(bpc=BPC, h=H, w=W, reps=1, wave_sizes=(4, 3), psum8=False,
                  drain="alt"):
    """Build the per-core SPMD program (identical on all cores).

    reps>1 unrolls the whole body N times (timing-only: lets wall-clock
    differences measure per-iteration device time past dispatch overhead).
    wave_sizes: psum-block grouping per conv_ot; each wave loads every
      weight tap once, so fewer waves = fewer PE weight loads.
    psum8: rotate psum tiles through all 8 banks (warm tile joins the pool).
    drain: "alt" (ACT/DVE alternating), "act", or "dve".
    """
    assert h % R == 0
    nblk = h // R
    assert sum(wave_sizes) == nblk
    n = R * w              # matmul free dim per block
    hp = h + 2             # padded rows (row 0 and hp-1 stay zero)
    wpr = w + 4            # padded row pitch (data at cols 2..w+1)
    hw = h * w

    nc = bacc.Bacc(
        "TRN2", target_bir_lowering=False, debug=False, enable_asserts=False
    )

    x_d = nc.declare_dram_parameter("x", [bpc, C_IN, h, w], F32, isOutput=False)
    wt_d = nc.declare_dram_parameter(
        "wt", [128, CT, 3, K_EXP, 3, C_OUT], BF16, isOutput=False
    )
    awt_d = nc.declare_dram_parameter("att_wt", [128, CT, K_EXP], F32, isOutput=False)
    bias_d = nc.declare_dram_parameter(
        "biasT", [128, OT, K_EXP], F32, isOutput=False
    )
    inv_d = nc.declare_dram_parameter("bninv", [C_OUT], F32, isOutput=False)
    cnst_d = nc.declare_dram_parameter("bncnst", [C_OUT], F32, isOutput=False)
    y_d = nc.declare_dram_parameter("y", [bpc, C_OUT, hw], F32, isOutput=True)

    with ExitStack() as ctx:
        tc = ctx.enter_context(tile.TileContext(nc))
        cpool = ctx.enter_context(tc.tile_pool(name="consts", bufs=1))
        wpool = ctx.enter_context(tc.tile_pool(name="work", bufs=2))
        ppool = ctx.enter_context(
            tc.tile_pool(
                name="psumc", bufs=8 if psum8 else nblk, space=bass.MemorySpace.PSUM
            )
        )
        if not psum8:
            spool = ctx.enter_context(
                tc.tile_pool(name="psums", bufs=1, space=bass.MemorySpace.PSUM)
            )
        else:
            spool = ppool

        # ---- per-sample state ----------------------------------------------
        pooled = cpool.tile([128, bpc * CT], F32, tag="pooled", name="pooled")
        att_bc = cpool.tile([128, bpc, K_EXP], F32, tag="attbc", name="att_bc")
        bias_tot = cpool.tile([128, bpc, OT], F32, tag="btot", name="bias_tot")
        xstage = {}
        xpad = {}
        aggs = {}
        atts = {}

        def prep_load(b):
            """x DMA (SWDGE ring) + cast-to-padded-bf16 + pooled sum."""
            for ct in range(CT):
                xs = wpool.tile(
                    [128, h, w], F32, tag="xstage", bufs=4, name=f"xs_{b}_{ct}"
                )
                nc.gpsimd.dma_start(out=xs[:], in_=x_d[b, ct * 128 : (ct + 1) * 128])
                xstage[b, ct] = xs
            for ct in range(CT):
                col = b * CT + ct
                nc.scalar.activation(
                    xpad[b, ct][:, 1 : h + 1, 2 : w + 2],
                    xstage[b, ct][:],
                    AF.Copy,
                    accum_out=pooled[:, col : col + 1],
                )

        # sample-0 x loads first (front of the DMA queues)
        for ct in range(CT):
            xs = wpool.tile([128, h, w], F32, tag="xstage", bufs=4, name=f"xs_0_{ct}")
            nc.gpsimd.dma_start(out=xs[:], in_=x_d[0, ct * 128 : (ct + 1) * 128])
            xstage[0, ct] = xs

        def prep_load0():
            """casts for sample 0 (x DMAs were issued up front)."""
            for ct in range(CT):
                nc.scalar.activation(
                    xpad[0, ct][:, 1 : h + 1, 2 : w + 2],
                    xstage[0, ct][:],
                    AF.Copy,
                    accum_out=pooled[:, ct : ct + 1],
                )

        # ---- resident constants --------------------------------------------
        # wt chunks follow sample-0's x on the same SWDGE ring so x lands
        # first; (ct, kh)-chunking lets the weight-combine (and the conv)
        # start before the whole bank has arrived.
        wt_sb = cpool.tile(
            [128, CT, 3, K_EXP, 3, C_OUT], BF16, tag="wt", name="wt_sb"
        )
        for ct, kh in product(range(CT), range(3)):
            nc.gpsimd.dma_start(out=wt_sb[:, ct, kh], in_=wt_d[:, ct, kh])
        awt_sb = cpool.tile([128, CT, K_EXP], F32, tag="awt", name="awt_sb")
        nc.sync.dma_start(out=awt_sb[:], in_=awt_d[:])
        bias_sb = cpool.tile([128, OT, K_EXP], F32, tag="bias", name="bias_sb")
        nc.sync.dma_start(out=bias_sb[:], in_=bias_d[:])
        inv_sb = cpool.tile([128, OT], F32, tag="inv", name="inv_sb")
        nc.sync.dma_start(out=inv_sb[:], in_=inv_d[:].rearrange("(t p) -> p t", p=128))
        cnst_sb = cpool.tile([128, OT], F32, tag="cnst", name="cnst_sb")
        nc.sync.dma_start(
            out=cnst_sb[:], in_=cnst_d[:].rearrange("(t p) -> p t", p=128)
        )
        ones_bf = cpool.tile([K_EXP, 128], BF16, tag="onesbf", name="ones_bf")
        nc.vector.memset(ones_bf[:], 1.0)

        for b, ct in product(range(bpc), range(CT)):
            t = cpool.tile(
                [128, hp, wpr], BF16, tag=f"xpad_{b}_{ct}", name=f"xpad_{b}_{ct}"
            )
            # zero only the borders (interior is fully overwritten);
            # all on GpSimd so the DVE stream stays clear for the combine
            nc.gpsimd.memset(t[:, 0:1, :], 0.0)
            nc.gpsimd.memset(t[:, hp - 1 : hp, :], 0.0)
            nc.gpsimd.memset(t[:, :, 0:2], 0.0)
            nc.gpsimd.memset(t[:, :, wpr - 2 : wpr], 0.0)
            xpad[b, ct] = t

        def prep_att_head(b):
            """attention for sample b, PE-free: per-partition products on
            DVE, cross-partition sum on GpSimd (result lands replicated on
            all partitions), sigmoid on ACT."""
            lg = wpool.tile([128, K_EXP], F32, tag="lgt", name=f"lgt_{b}")
            nc.vector.tensor_scalar_mul(
                lg[:], awt_sb[:, 0, :], pooled[:, b * CT : b * CT + 1]
            )
            nc.vector.scalar_tensor_tensor(
                lg[:],
                awt_sb[:, 1, :],
                pooled[:, b * CT + 1 : b * CT + 2],
                lg[:],
                op0=ALU.mult,
                op1=ALU.add,
            )
            red = wpool.tile([128, K_EXP], F32, tag="lgr", name=f"lgr_{b}")
            nc.gpsimd.partition_all_reduce(
                red[:], lg[:], 128, bass_isa.ReduceOp.add
            )
            nc.scalar.activation(att_bc[:, b, :], red[:], AF.Sigmoid, scale=1.0 / hw)

        def prep_att_combine(b):
            # agg = sum_k att[k] * wt[k], sub-chunked by (ct, kh) in conv
            # tap order, all on DVE (ts_mul runs 4x, tensor_add 2x bf16).
            # One tile per chunk keeps the dep tracking fine-grained: the
            # conv can start as soon as its first chunk is combined.
            agg = {}
            for ct, kh in product(range(CT), range(3)):
                a = wpool.tile(
                    [128, 3, C_OUT], BF16, tag="agg", bufs=12,
                    name=f"agg_{b}_{ct}{kh}",
                )
                nc.vector.tensor_scalar_mul(
                    a[:], wt_sb[:, ct, kh, 0], att_bc[:, b, 0:1]
                )
                for k in range(1, K_EXP):
                    tm = wpool.tile(
                        [128, 3, C_OUT], BF16, tag="tm", bufs=2,
                        name=f"tm_{b}{ct}{kh}{k}",
                    )
                    nc.vector.tensor_scalar_mul(
                        tm[:], wt_sb[:, ct, kh, k], att_bc[:, b, k : k + 1]
                    )
                    nc.vector.tensor_add(a[:], a[:], tm[:])
                agg[ct, kh] = a
            aggs[b] = agg

        def conv_ot(b, ot):
            """one output-channel tile of the conv for sample b."""
            agg = aggs[b]
            # bias_tot[o, b] = (sum_k att[k] bias[k, o]) * inv + cnst —
            # PE-free; only the drains (~25us later) need it
            pb = wpool.tile([128, K_EXP], F32, tag="pb", bufs=2, name=f"pb_{b}_{ot}")
            nc.vector.tensor_mul(pb[:], bias_sb[:, ot, :], att_bc[:, b, :])
            pbr = wpool.tile([128, 1], F32, tag="pbr", bufs=2, name=f"pbr_{b}_{ot}")
            nc.vector.tensor_reduce(
                pbr[:], pb[:], axis=mybir.AxisListType.X, op=ALU.add
            )
            nc.vector.tensor_scalar(
                bias_tot[:, b, ot : ot + 1],
                pbr[:],
                inv_sb[:, ot : ot + 1],
                cnst_sb[:, ot : ot + 1],
                op0=ALU.mult,
                op1=ALU.add,
            )
            taps = list(product(range(CT), range(3), range(3)))
            # waves of psum blocks: each wave streams every tap once, so a
            # wave's drains overlap the next wave's accumulation
            waves = []
            s = 0
            for wsz in wave_sizes:
                waves.append(list(range(s, s + wsz)))
                s += wsz
            for wave in waves:
                if not wave:
                    continue
                ps = {
                    blk: ppool.tile([128, n], F32, tag="cps", name=f"ps_{b}_{ot}_{blk}")
                    for blk in wave
                }
                for ci, (ct, kh, kw) in enumerate(taps):
                    lhsT = agg[ct, kh][:, kw, ot * 128 : (ot + 1) * 128]
                    for blk in wave:
                        nc.tensor.matmul(
                            ps[blk][:],
                            lhsT,
                            xpad[b, ct][
                                :, blk * R + kh : blk * R + kh + R, 1 + kw : 1 + kw + w
                            ],
                            start=(ci == 0),
                            stop=(ci == len(taps) - 1),
                        )
                # drain: relu(psum + bias_tot), engine per `drain` mode
                for blk in wave:
                    osb = wpool.tile(
                        [128, n], F32, tag="osb", bufs=6, name=f"osb_{b}_{ot}_{blk}"
                    )
                    on_act = blk % 2 == 0 if drain == "alt" else drain == "act"
                    if on_act:
                        nc.scalar.activation(
                            osb[:],
                            ps[blk][:],
                            AF.Relu,
                            bias=bias_tot[:, b, ot : ot + 1],
                        )
                    else:
                        nc.vector.tensor_scalar(
                            osb[:],
                            ps[blk][:],
                            bias_tot[:, b, ot : ot + 1],
                            0.0,
                            op0=ALU.add,
                            op1=ALU.max,
                        )
                    nc.sync.dma_start(
                        out=y_d[b, ot * 128 : (ot + 1) * 128, blk * n : (blk + 1) * n],
                        in_=osb[:],
                    )

        # ---- main schedule --------------------------------------------------
        for rep in range(reps):
            if rep == 0:
                prep_load0()
            else:
                prep_load(0)

            # PE warm-up: junk matmuls (gated on the first cast) keep HAM
            # from clocking the PE at 1.2 GHz at the first conv matmuls.
            if rep == 0:
                warm = spool.tile(
                    [128, n], F32, tag="cps" if psum8 else "sps", name="warm_ps"
                )
                for i in range(8):
                    nc.tensor.matmul(
                        warm[:],
                        ones_bf[:, 0:128],
                        xpad[0, 0][0:K_EXP, 1 : 1 + R, 2 : 2 + w],
                        start=True,
                        stop=True,
                    )

            prep_att_head(0)
            prep_att_combine(0)
            for b in range(bpc):
                if b + 1 < bpc:
                    prep_load(b + 1)
                    prep_att_head(b + 1)
                    prep_att_combine(b + 1)
                conv_ot(b, 0)
                conv_ot(b, 1)
    nc.compile()
    return nc


def host_inputs(inputs, bpc=BPC, h=H, w=W):
    """Shard x over batch; lay out replicated params for the device."""
    x = np.ascontiguousarray(np.asarray(inputs["x"], dtype=np.float32))
    att_w = np.asarray(inputs["att_w"], dtype=np.float32)
    weight = np.asarray(inputs["weight"], dtype=np.float32)
    bias = np.asarray(inputs["bias"], dtype=np.float32)

    # Fold the BN scale inv = gamma/sqrt(var+eps) into the conv weights so
    # the PSUM drain is a single relu(psum + bias_tot) op.  (bias_tot gets
    # its inv factor on-device.)
    inv = np.asarray(inputs["gamma"], dtype=np.float32) / np.sqrt(
        np.asarray(inputs["run_var"], dtype=np.float32) + 1e-5
    )
    cnst = np.asarray(inputs["beta"], dtype=np.float32) - (
        np.asarray(inputs["run_mean"], dtype=np.float32) * inv
    )
    weight = weight * inv[None, :, None, None, None]

    # wt[i_lo, ct, kh, k, kw, o] = weight[k, o, ct*128+i_lo, kh, kw]
    wt = weight.reshape(K_EXP, C_OUT, CT, 128, 3, 3)
    wt = wt.transpose(3, 2, 4, 0, 5, 1).reshape(128, CT, 3, K_EXP, 3, C_OUT)
    wt = np.ascontiguousarray(wt).astype(ml_dtypes.bfloat16)
    # att_wt[c_lo, ct, k] = att_w[k, ct*128+c_lo]
    awt = np.ascontiguousarray(
        att_w.T.reshape(CT, 128, K_EXP).transpose(1, 0, 2)
    ).astype(np.float32)

    biasT = np.ascontiguousarray(
        bias.T.reshape(OT, 128, K_EXP).transpose(1, 0, 2)
    ).astype(np.float32)
    common = {
        "wt": wt,
        "att_wt": awt,
        "biasT": biasT,
        "bninv": inv.astype(np.float32),
        "bncnst": cnst.astype(np.float32),
    }
    return [
        {"x": x[c * bpc : (c + 1) * bpc], **common} for c in range(x.shape[0] // bpc)
    ]


_CACHE = {}


def _program():
    if "nc" not in _CACHE:
        _CACHE["nc"] = build_program()
    return _CACHE["nc"]


def run(inputs, trace=False, **kw):
    nc = _program()
    in_maps = host_inputs(inputs)
    res = run_bass_kernel_spmd(nc, in_maps, list(range(NCORES)), trace=trace, **kw)
    y = np.concatenate(
        [res.results[c]["y"].reshape(BPC, C_OUT, H, W) for c in range(NCORES)], axis=0
    )
    return np.ascontiguousarray(y.astype(np.float32)), res


def kernel(**inputs):
    y, _ = run(inputs)
    return y

